# revision 11
# baseline (speedup 1.0000x reference)
"""Trainium2 Bass kernel for CBSA (cross-block self-attention) module.

Shapes (hardcoded from the problem spec):
  x: [8, 4096, 512], proj_w/to_out_w: [512, 512], step_rep/step_x: [8,1,1],
  to_out_b: [512].  Output: [8, 4096, 512].

Sharding: data-parallel over batch, 1 batch per NeuronCore (8 cores).
"""

import numpy as np
import ml_dtypes

import concourse.bass as bass
import concourse.tile as tile
from concourse import bacc, mybir
from concourse import bass_utils

F32 = mybir.dt.float32
F32R = mybir.dt.float32r
BF16 = mybir.dt.bfloat16

B = 8
N = 4096
C = 512
HEADS = 8
DH = 64
Q = 64            # pooled tokens
SCALE = DH ** -0.5
NT = N // 128     # 32 token tiles
CH = C // 128     # 4 feature chunks
PAIRS = HEADS // 2  # 4 head pairs
NS = N // 512     # 8 free-dim slices of 512

_CACHE = {}
DEBUG = False


def _build():
    nc = bacc.Bacc("TRN2", target_bir_lowering=False, debug=False, num_devices=B)

    xT_d = nc.dram_tensor("xT", [128, CH, N], F32R, kind="ExternalInput").ap()
    pwT_d = nc.dram_tensor("pwT", [128, CH, C], F32R, kind="ExternalInput").ap()
    twT_d = nc.dram_tensor("twT", [128, CH, C], BF16, kind="ExternalInput").ap()
    bias_d = nc.dram_tensor("bias", [128, C], F32, kind="ExternalInput").ap()
    mp_d = nc.dram_tensor("mpool", [128, NT, Q], BF16, kind="ExternalInput").ap()
    srep_d = nc.dram_tensor("srep", [128, PAIRS], F32, kind="ExternalInput").ap()
    idf_d = nc.dram_tensor("identf", [128, 128], F32, kind="ExternalInput").ap()
    idb_d = nc.dram_tensor("identb", [128, 128], BF16, kind="ExternalInput").ap()
    out_d = nc.dram_tensor("out", [N, C], F32, kind="ExternalOutput").ap()
    taps = {}
    if DEBUG:
        def tapdecl(name, shape, dt):
            taps[name] = nc.dram_tensor("tap_" + name, shape, dt,
                                        kind="ExternalOutput").ap()
        tapdecl("wtb0", [128, N], BF16)
        tapdecl("w0", [128, C], BF16)
        tapdecl("rep", [Q, C], F32)
        tapdecl("rep_pair", [128, PAIRS * DH], F32)
        tapdecl("dblk0", [128, 128], BF16)
        tapdecl("ed0", [128, N], BF16)
        tapdecl("s10", [128, 1], F32)
        tapdecl("at0", [128, 128], BF16)
        tapdecl("rd0", [128, 128], F32)
        tapdecl("rnat0", [128, 128], BF16)
        tapdecl("rnT0", [128, 128], BF16)
        tapdecl("ed20", [128, 128], BF16)
        tapdecl("xds0", [128, 128], BF16)
        tapdecl("xdT0", [128, N], BF16)

    from contextlib import ExitStack
    with tile.TileContext(nc) as tc:
        with ExitStack() as ctx:
            _body.ctx = ctx
            _body(tc, nc, xT_d, pwT_d, twT_d, bias_d, mp_d, srep_d, idf_d, idb_d,
                  out_d, taps)
    nc.compile()
    return nc


def _body(tc, nc, xT_d, pwT_d, twT_d, bias_d, mp_d, srep_d, idf_d, idb_d, out_d,
          taps=None):
    def tap(name, ap):
        if taps and name in taps:
            nc.sync.dma_start(taps[name][:], ap)
    Exp = mybir.ActivationFunctionType.Exp
    X = mybir.AxisListType.X
    mult = mybir.AluOpType.add  # placeholder; real ops referenced inline

    ctx = _body.ctx
    const = ctx.enter_context(tc.tile_pool(name="const", bufs=1))
    persist = ctx.enter_context(tc.tile_pool(name="persist", bufs=1))
    xs_pool = ctx.enter_context(tc.tile_pool(name="xstream", bufs=10))
    ed_pool = ctx.enter_context(tc.tile_pool(name="ed", bufs=2))
    at_pool = ctx.enter_context(tc.tile_pool(name="at", bufs=2))
    sm_pool = ctx.enter_context(tc.tile_pool(name="small", bufs=3))
    ost_pool = ctx.enter_context(tc.tile_pool(name="ostage", bufs=4))
    ps512 = ctx.enter_context(tc.tile_pool(name="ps512", bufs=3, space="PSUM"))
    ps128 = ctx.enter_context(tc.tile_pool(name="ps128", bufs=2, space="PSUM"))
    psrep = ctx.enter_context(tc.tile_pool(name="psrep", bufs=1, space="PSUM"))

    # ---- constants / small inputs ----
    pwT = const.tile([128, CH, C], F32R, tag="pwT")
    nc.sync.dma_start(pwT[:], pwT_d[:])
    twT = const.tile([128, CH, C], BF16, tag="twT")
    nc.sync.dma_start(twT[:], twT_d[:])
    bias = const.tile([128, C], F32, tag="bias")
    nc.sync.dma_start(bias[:], bias_d[:])
    mpool = const.tile([128, NT, Q], BF16, tag="mpool")
    nc.sync.dma_start(mpool[:], mp_d[:])
    srep = const.tile([128, PAIRS], F32, tag="srep")
    nc.sync.dma_start(srep[:], srep_d[:])
    identf = const.tile([128, 128], F32, tag="identf")
    nc.sync.dma_start(identf[:], idf_d[:])
    identb = const.tile([128, 128], BF16, tag="identb")
    nc.sync.dma_start(identb[:], idb_d[:])

    # ---- persistent intermediates ----
    # wtb[di]: wT chunk di in bf16, [128 (d local), N]
    wtb = [persist.tile([128, N], BF16, tag=f"wtb{di}", name=f"wtb{di}")
           for di in range(CH)]
    # w natural, bf16: [128 (n local), NT, C]
    w_sb = persist.tile([128, NT, C], BF16, tag="w_sb")
    # x_deltaT chunks, bf16
    xdT = [persist.tile([128, N], BF16, tag=f"xdT{di}", name=f"xdT{di}")
           for di in range(CH)]

    # ================= Phase 1: wT = proj_w @ x^T  =================
    # out[d, n] = sum_c proj_w[d, c] x[n, c]; lhsT = pwT[ci][:, di*128:...],
    # rhs = xT[ci][:, s*512:...] streamed from DRAM.
    SB = 2  # psum slices in flight
    for di in range(CH):
        for sb in range(NS // SB):
            pst = [ps512.tile([128, 512], F32, tag="ps512", name=f"pst{s2_}")
                   for s2_ in range(SB)]
            for ci in range(CH):
                xt = [xs_pool.tile([128, 512], F32R, tag="xs", name=f"xt{s2_}")
                      for s2_ in range(SB)]
                for s2 in range(SB):
                    s = sb * SB + s2
                    nc.sync.dma_start(xt[s2][:], xT_d[:, ci, s * 512:(s + 1) * 512])
                    nc.tensor.matmul(
                        pst[s2][:],
                        pwT[:, ci, di * 128:(di + 1) * 128],
                        xt[s2][:],
                        start=(ci == 0), stop=(ci == CH - 1),
                    )
            for s2 in range(SB):
                s = sb * SB + s2
                nc.scalar.copy(wtb[di][:, s * 512:(s + 1) * 512], pst[s2][:])

    tap("wtb0", wtb[0][:])
    # ================= Phase 2: w natural via DMA xbar transposes ====
    for di in range(CH):
        for t in range(NT):
            nc.sync.dma_start_transpose(
                w_sb[:, t, di * 128:(di + 1) * 128],
                wtb[di][:, t * 128:(t + 1) * 128],
            )

    # ================= Phase 3: pooled rep =================
    rep_ps = psrep.tile([Q, C], F32, tag="rep")
    for t in range(NT):
        nc.tensor.matmul(rep_ps[:], mpool[:, t, :], w_sb[:, t, :],
                         start=(t == 0), stop=(t == NT - 1))
    rep = sm_pool.tile([Q, C], F32, tag="rep_sb")
    nc.vector.tensor_copy(rep[:], rep_ps[:])

    tap("w0", w_sb[:, 0, :])
    tap("rep", rep[:])
    # rep_pair[qp, p, dh]: rows 0:64 = head 2p queries, 64:128 = head 2p+1.
    # Built with identity matmuls (tile_position moves partitions).
    rp_ps = ps128.tile([128, PAIRS * DH], F32, tag="ps128")
    for p in range(PAIRS):
        nc.tensor.matmul(rp_ps[0:64, p * DH:(p + 1) * DH], identf[0:Q, 0:Q],
                         rep[:, (2 * p) * DH:(2 * p + 1) * DH],
                         start=True, stop=True)
        nc.tensor.matmul(rp_ps[64:128, p * DH:(p + 1) * DH], identf[0:Q, 0:Q],
                         rep[:, (2 * p + 1) * DH:(2 * p + 2) * DH],
                         start=True, stop=True)
    rep_pair = sm_pool.tile([128, PAIRS, DH], F32, tag="rep_pair")
    nc.vector.tensor_copy(rep_pair.rearrange("p a b -> p (a b)")[:], rp_ps[:])

    tap("rep_pair", rep_pair.rearrange("p a b -> p (a b)")[:])
    # repT chunks + block-diag lhsT for dots (bf16)
    dblk = []
    for p in range(PAIRS):
        tp = ps128.tile([128, Q], F32, tag="ps128")
        nc.tensor.transpose(tp[:], rep[:, p * 128:(p + 1) * 128], identf[0:Q, 0:Q])
        bk = sm_pool.tile([128, 128], BF16, tag=f"dblk{p}")
        nc.vector.memset(bk[:], 0.0)
        nc.vector.tensor_copy(bk[0:64, 0:64], tp[0:64, :])
        nc.vector.tensor_copy(bk[64:128, 64:128], tp[64:128, :])
        dblk.append(bk)

    tap("dblk0", dblk[0][:])
    # ================= Phase 4: per head-pair attention =================
    for p in range(PAIRS):
        # --- dots + exp + row sums ---
        ed = ed_pool.tile([128, N], BF16, tag="ed")
        s1parts = sm_pool.tile([128, NS], F32, tag="s1parts")
        for s in range(NS):
            dps = ps512.tile([128, 512], F32, tag="ps512")
            nc.tensor.matmul(dps[:], dblk[p][:], wtb[p][:, s * 512:(s + 1) * 512],
                             start=True, stop=True)
            nc.scalar.activation(ed[:, s * 512:(s + 1) * 512], dps[:], Exp,
                                 scale=SCALE, accum_out=s1parts[:, s:s + 1])
        s1 = sm_pool.tile([128, 1], F32, tag="s1")
        nc.vector.tensor_reduce(s1[:], s1parts[:], X, mybir.AluOpType.add)
        rc1 = sm_pool.tile([128, 1], F32, tag="rc1")
        nc.vector.reciprocal(rc1[:], s1[:])
        ssc = sm_pool.tile([128, 1], F32, tag="ssc")
        nc.vector.tensor_mul(ssc[:], rc1[:], srep[:, p:p + 1])
        if p == 0:
            tap("ed0", ed[:])
            tap("s10", s1[:])

        # --- attnT via DMA xbar transposes ---
        at = at_pool.tile([128, NT, 128], BF16, tag="at")
        for t in range(NT):
            nc.sync.dma_start_transpose(at[:, t, :], ed[:, t * 128:(t + 1) * 128])

        # --- rep_delta[qpair, d-block p] ---
        rd_ps = ps128.tile([128, 128], F32, tag="ps128")
        for t in range(NT):
            nc.tensor.matmul(rd_ps[:], at[:, t, :], w_sb[:, t, p * 128:(p + 1) * 128],
                             start=(t == 0), stop=(t == NT - 1))

        if p == 0:
            tap("at0", at[:, 0, :])
            rd_tap = sm_pool.tile([128, 128], F32, tag="rd_tap")
            nc.vector.tensor_copy(rd_tap[:], rd_ps[:])
            tap("rd0", rd_tap[:])
        # --- reph_new (block-diag, natural layout) ---
        rnat = sm_pool.tile([128, 128], BF16, tag="rnat")
        nc.vector.memset(rnat[:], 0.0)
        for h in range(2):
            r0, r1 = 64 * h, 64 * (h + 1)
            nc.vector.scalar_tensor_tensor(
                rnat[r0:r1, r0:r1], rd_ps[r0:r1, r0:r1], ssc[r0:r1, 0:1],
                rep_pair[r0:r1, p, :],
                mybir.AluOpType.mult, mybir.AluOpType.add)

        # --- reph_new^T via DMA xbar transpose ---
        rnT = sm_pool.tile([128, 128], BF16, tag="rnT")
        nc.sync.dma_start_transpose(rnT[:], rnat[:])

        if p == 0:
            tap("rnat0", rnat[:])
            tap("rnT0", rnT[:])
        # --- dots2 (block-diag, symmetric) + exp + sums ---
        d2_ps = ps128.tile([128, 128], F32, tag="ps128")
        nc.tensor.matmul(d2_ps[:], rnT[:], rnT[:], start=True, stop=True)
        ed2 = sm_pool.tile([128, 128], BF16, tag="ed2")
        nc.vector.memset(ed2[:], 0.0)
        s2 = sm_pool.tile([128, 1], F32, tag="s2")
        for h in range(2):
            r0, r1 = 64 * h, 64 * (h + 1)
            nc.scalar.activation(ed2[r0:r1, r0:r1], d2_ps[r0:r1, r0:r1], Exp,
                                 scale=SCALE, accum_out=s2[r0:r1, 0:1])

        # --- xds = attn2 @ reph_new, then scale rows by 1/(s1*s2) ---
        xds_ps = ps128.tile([128, 128], F32, tag="ps128")
        nc.tensor.matmul(xds_ps[:], ed2[:], rnat[:], start=True, stop=True)
        rc2 = sm_pool.tile([128, 1], F32, tag="rc2")
        nc.vector.reciprocal(rc2[:], s2[:])
        sc = sm_pool.tile([128, 1], F32, tag="sc")
        nc.vector.tensor_mul(sc[:], rc1[:], rc2[:])
        xds = sm_pool.tile([128, 128], BF16, tag="xds")
        nc.vector.tensor_scalar_mul(xds[:], xds_ps[:], sc[:])

        if p == 0:
            tap("ed20", ed2[:])
            tap("xds0", xds[:])
        # --- upsample: x_deltaT[d-pair, n] = xds^T @ expdots ---
        for s in range(NS):
            up_ps = ps512.tile([128, 512], F32, tag="ps512")
            nc.tensor.matmul(up_ps[:], xds[:], ed[:, s * 512:(s + 1) * 512],
                             start=True, stop=True)
            nc.vector.tensor_copy(xdT[p][:, s * 512:(s + 1) * 512], up_ps[:])

    tap("xdT0", xdT[0][:])
    # ================= Phase 5: out = x_delta @ to_out_w^T + b =========
    for t in range(NT):
        ops = ps512.tile([128, 512], F32, tag="ps512")
        for di in range(CH):
            nc.tensor.matmul(ops[:], xdT[di][:, t * 128:(t + 1) * 128], twT[:, di, :],
                             start=(di == 0), stop=(di == CH - 1))
        ot = ost_pool.tile([128, C], F32, tag="ostage")
        nc.vector.tensor_add(ot[:], ops[:], bias[:])
        nc.sync.dma_start(out_d[t * 128:(t + 1) * 128, :], ot[:])


def _prep_inputs(x, proj_w, step_rep, step_x, to_out_w, to_out_b):
    x = np.asarray(x, dtype=np.float32)
    proj_w = np.asarray(proj_w, dtype=np.float32)
    step_rep = np.asarray(step_rep, dtype=np.float32).reshape(HEADS)
    step_x = np.asarray(step_x, dtype=np.float32).reshape(HEADS)
    to_out_w = np.asarray(to_out_w, dtype=np.float32)
    to_out_b = np.asarray(to_out_b, dtype=np.float32)

    pwT = np.ascontiguousarray(proj_w.T.reshape(CH, 128, C).transpose(1, 0, 2))
    twTs = np.ascontiguousarray(to_out_w.T) * np.repeat(step_x, DH)[:, None]
    twTs = np.ascontiguousarray(
        twTs.reshape(CH, 128, C).transpose(1, 0, 2)).astype(ml_dtypes.bfloat16)
    bias = np.broadcast_to(to_out_b, (128, C)).copy()

    # pooling matrix M[n, q] = 1/64 for members, layout [128, NT, Q]
    n_idx = np.arange(N)
    h_idx, w_idx = n_idx // 64, n_idx % 64
    q_idx = (h_idx // 8) * 8 + (w_idx // 8)
    M = np.zeros((N, Q), dtype=np.float32)
    M[n_idx, q_idx] = 1.0 / 64.0
    mp = M.reshape(NT, 128, Q).transpose(1, 0, 2).astype(ml_dtypes.bfloat16)
    mp = np.ascontiguousarray(mp)

    srep = np.empty((128, PAIRS), dtype=np.float32)
    for p in range(PAIRS):
        srep[0:64, p] = step_rep[2 * p]
        srep[64:128, p] = step_rep[2 * p + 1]

    identf = np.eye(128, dtype=np.float32)
    identb = np.eye(128, dtype=ml_dtypes.bfloat16)

    shared = {
        "pwT": pwT, "twT": twTs, "bias": bias, "mpool": mp,
        "srep": srep, "identf": identf, "identb": identb,
    }
    in_maps = []
    for b in range(B):
        xT = np.ascontiguousarray(x[b].T.reshape(CH, 128, N).transpose(1, 0, 2))
        in_maps.append({"xT": xT, **shared})
    return in_maps


def kernel(x, proj_w, step_rep, step_x, to_out_w, to_out_b):
    if "nc" not in _CACHE:
        _CACHE["nc"] = _build()
    nc = _CACHE["nc"]
    in_maps = _prep_inputs(x, proj_w, step_rep, step_x, to_out_w, to_out_b)
    res = bass_utils.run_bass_kernel_spmd(nc, in_maps, core_ids=list(range(B)))
    return np.stack([res.results[b]["out"] for b in range(B)], axis=0)


# revision 13
# speedup vs baseline: 2.0178x; 2.0178x over previous
"""Trainium2 Bass kernel for CBSA (cross-block self-attention) module.

Shapes (hardcoded from the problem spec):
  x: [8, 4096, 512], proj_w/to_out_w: [512, 512], step_rep/step_x: [8,1,1],
  to_out_b: [512].  Output: [8, 4096, 512].

Sharding: data-parallel over batch, 1 batch per NeuronCore (8 cores).
"""

import numpy as np
import ml_dtypes

import concourse.bass as bass
import concourse.tile as tile
from concourse import bacc, mybir
from concourse import bass_utils

F32 = mybir.dt.float32
F32R = mybir.dt.float32r
BF16 = mybir.dt.bfloat16

B = 8
N = 4096
C = 512
HEADS = 8
DH = 64
Q = 64            # pooled tokens
SCALE = DH ** -0.5
NT = N // 128     # 32 token tiles
CH = C // 128     # 4 feature chunks
PAIRS = HEADS // 2  # 4 head pairs
NS = N // 512     # 8 free-dim slices of 512

_CACHE = {}
DEBUG = False


def _build():
    nc = bacc.Bacc("TRN2", target_bir_lowering=False, debug=False, num_devices=B)

    xT_d = nc.dram_tensor("xT", [128, CH, N], F32R, kind="ExternalInput").ap()
    pwT_d = nc.dram_tensor("pwT", [128, CH, C], F32R, kind="ExternalInput").ap()
    twT_d = nc.dram_tensor("twT", [128, CH, C], BF16, kind="ExternalInput").ap()
    bias_d = nc.dram_tensor("bias", [128, C], F32, kind="ExternalInput").ap()
    mp_d = nc.dram_tensor("mpool", [128, NT, Q], BF16, kind="ExternalInput").ap()
    srep_d = nc.dram_tensor("srep", [128, PAIRS], F32, kind="ExternalInput").ap()
    idf_d = nc.dram_tensor("identf", [128, 128], F32, kind="ExternalInput").ap()
    idb_d = nc.dram_tensor("identb", [128, 128], BF16, kind="ExternalInput").ap()
    out_d = nc.dram_tensor("out", [N, C], F32, kind="ExternalOutput").ap()
    taps = {}
    if DEBUG:
        def tapdecl(name, shape, dt):
            taps[name] = nc.dram_tensor("tap_" + name, shape, dt,
                                        kind="ExternalOutput").ap()
        tapdecl("wtb0", [128, N], BF16)
        tapdecl("w0", [128, C], BF16)
        tapdecl("rep", [Q, C], F32)
        tapdecl("rep_pair", [128, PAIRS * DH], F32)
        tapdecl("dblk0", [128, 128], BF16)
        tapdecl("ed0", [128, N], BF16)
        tapdecl("s10", [128, 1], F32)
        tapdecl("at0", [128, 128], BF16)
        tapdecl("rd0", [128, 128], F32)
        tapdecl("rnat0", [128, 128], BF16)
        tapdecl("rnT0", [128, 128], BF16)
        tapdecl("ed20", [128, 128], BF16)
        tapdecl("xds0", [128, 128], BF16)
        tapdecl("xdT0", [128, N], BF16)

    from contextlib import ExitStack
    with tile.TileContext(nc) as tc:
        with ExitStack() as ctx:
            _body.ctx = ctx
            _body(tc, nc, xT_d, pwT_d, twT_d, bias_d, mp_d, srep_d, idf_d, idb_d,
                  out_d, taps)
    nc.compile()
    return nc


def _body(tc, nc, xT_d, pwT_d, twT_d, bias_d, mp_d, srep_d, idf_d, idb_d, out_d,
          taps=None):
    def tap(name, ap):
        if taps and name in taps:
            nc.sync.dma_start(taps[name][:], ap)
    Exp = mybir.ActivationFunctionType.Exp
    X = mybir.AxisListType.X
    mult = mybir.AluOpType.add  # placeholder; real ops referenced inline

    ctx = _body.ctx
    const = ctx.enter_context(tc.tile_pool(name="const", bufs=1))
    persist = ctx.enter_context(tc.tile_pool(name="persist", bufs=1))
    xs_pool = ctx.enter_context(tc.tile_pool(name="xstream", bufs=10))
    ed_pool = ctx.enter_context(tc.tile_pool(name="ed", bufs=2))
    at_pool = ctx.enter_context(tc.tile_pool(name="at", bufs=2))
    sm_pool = ctx.enter_context(tc.tile_pool(name="small", bufs=3))
    ost_pool = ctx.enter_context(tc.tile_pool(name="ostage", bufs=4))
    ps512 = ctx.enter_context(tc.tile_pool(name="ps512", bufs=3, space="PSUM"))
    ps128 = ctx.enter_context(tc.tile_pool(name="ps128", bufs=2, space="PSUM"))
    psrep = ctx.enter_context(tc.tile_pool(name="psrep", bufs=1, space="PSUM"))
    pstr = ctx.enter_context(tc.tile_pool(name="pstr", bufs=2, space="PSUM"))

    # ---- constants / small inputs ----
    pwT = const.tile([128, CH, C], F32R, tag="pwT")
    nc.sync.dma_start(pwT[:], pwT_d[:])
    twT = const.tile([128, CH, C], BF16, tag="twT")
    nc.sync.dma_start(twT[:], twT_d[:])
    bias = const.tile([128, C], F32, tag="bias")
    nc.sync.dma_start(bias[:], bias_d[:])
    mpool = const.tile([128, NT, Q], BF16, tag="mpool")
    nc.sync.dma_start(mpool[:], mp_d[:])
    srep = const.tile([128, PAIRS], F32, tag="srep")
    nc.sync.dma_start(srep[:], srep_d[:])
    identf = const.tile([128, 128], F32, tag="identf")
    nc.sync.dma_start(identf[:], idf_d[:])
    identb = const.tile([128, 128], BF16, tag="identb")
    nc.sync.dma_start(identb[:], idb_d[:])

    # ---- persistent intermediates ----
    # wtb[di]: wT chunk di in bf16, [128 (d local), N]
    wtb = [persist.tile([128, N], BF16, tag=f"wtb{di}", name=f"wtb{di}")
           for di in range(CH)]
    # w natural, bf16: [128 (n local), NT, C]
    w_sb = persist.tile([128, NT, C], BF16, tag="w_sb")
    # x_deltaT chunks, bf16
    xdT = [persist.tile([128, N], BF16, tag=f"xdT{di}", name=f"xdT{di}")
           for di in range(CH)]

    # ================= Phase 1: wT = proj_w @ x^T  =================
    # out[d, n] = sum_c proj_w[d, c] x[n, c]; lhsT = pwT[ci][:, di*128:...],
    # rhs = xT[ci][:, s*512:...] streamed from DRAM.
    SB = 2  # psum slices in flight
    for di in range(CH):
        for sb in range(NS // SB):
            pst = [ps512.tile([128, 512], F32, tag="ps512", name=f"pst{s2_}")
                   for s2_ in range(SB)]
            for ci in range(CH):
                xt = [xs_pool.tile([128, 512], F32R, tag="xs", name=f"xt{s2_}")
                      for s2_ in range(SB)]
                for s2 in range(SB):
                    s = sb * SB + s2
                    nc.sync.dma_start(xt[s2][:], xT_d[:, ci, s * 512:(s + 1) * 512])
                    nc.tensor.matmul(
                        pst[s2][:],
                        pwT[:, ci, di * 128:(di + 1) * 128],
                        xt[s2][:],
                        start=(ci == 0), stop=(ci == CH - 1),
                    )
            for s2 in range(SB):
                s = sb * SB + s2
                nc.scalar.copy(wtb[di][:, s * 512:(s + 1) * 512], pst[s2][:])

    tap("wtb0", wtb[0][:])
    # ================= Phase 2: w natural via PE transposes ====
    for di in range(CH):
        for t in range(NT):
            wtp = pstr.tile([128, 128], BF16, tag="pstr", name="wtp")
            nc.tensor.transpose(wtp[:], wtb[di][:, t * 128:(t + 1) * 128],
                                identb[:])
            eng = nc.vector if (t % 2 == 0) else nc.scalar
            if eng is nc.vector:
                eng.tensor_copy(w_sb[:, t, di * 128:(di + 1) * 128], wtp[:])
            else:
                eng.copy(w_sb[:, t, di * 128:(di + 1) * 128], wtp[:])

    # ================= Phase 3: pooled rep =================
    rep_ps = psrep.tile([Q, C], F32, tag="rep")
    for t in range(NT):
        nc.tensor.matmul(rep_ps[:], mpool[:, t, :], w_sb[:, t, :],
                         start=(t == 0), stop=(t == NT - 1))
    rep = sm_pool.tile([Q, C], F32, tag="rep_sb")
    nc.vector.tensor_copy(rep[:], rep_ps[:])

    tap("w0", w_sb[:, 0, :])
    tap("rep", rep[:])
    # rep_pair[qp, p, dh]: rows 0:64 = head 2p queries, 64:128 = head 2p+1.
    # Built with identity matmuls (tile_position moves partitions).
    rp_ps = ps128.tile([128, PAIRS * DH], F32, tag="ps128")
    for p in range(PAIRS):
        nc.tensor.matmul(rp_ps[0:64, p * DH:(p + 1) * DH], identf[0:Q, 0:Q],
                         rep[:, (2 * p) * DH:(2 * p + 1) * DH],
                         start=True, stop=True)
        nc.tensor.matmul(rp_ps[64:128, p * DH:(p + 1) * DH], identf[0:Q, 0:Q],
                         rep[:, (2 * p + 1) * DH:(2 * p + 2) * DH],
                         start=True, stop=True)
    rep_pair = sm_pool.tile([128, PAIRS, DH], F32, tag="rep_pair")
    nc.vector.tensor_copy(rep_pair.rearrange("p a b -> p (a b)")[:], rp_ps[:])

    tap("rep_pair", rep_pair.rearrange("p a b -> p (a b)")[:])
    # repT chunks + block-diag lhsT for dots (bf16)
    dblk = []
    for p in range(PAIRS):
        tp = ps128.tile([128, Q], F32, tag="ps128")
        nc.tensor.transpose(tp[:], rep[:, p * 128:(p + 1) * 128], identf[0:Q, 0:Q])
        bk = sm_pool.tile([128, 128], BF16, tag=f"dblk{p}")
        nc.vector.memset(bk[:], 0.0)
        nc.vector.tensor_copy(bk[0:64, 0:64], tp[0:64, :])
        nc.vector.tensor_copy(bk[64:128, 64:128], tp[64:128, :])
        dblk.append(bk)

    tap("dblk0", dblk[0][:])
    # ================= Phase 4: per head-pair attention =================
    for p in range(PAIRS):
        # --- dots + exp + row sums ---
        ed = ed_pool.tile([128, N], BF16, tag="ed")
        s1parts = sm_pool.tile([128, NS], F32, tag="s1parts")
        for s in range(NS):
            dps = ps512.tile([128, 512], F32, tag="ps512")
            nc.tensor.matmul(dps[:], dblk[p][:], wtb[p][:, s * 512:(s + 1) * 512],
                             start=True, stop=True)
            nc.scalar.activation(ed[:, s * 512:(s + 1) * 512], dps[:], Exp,
                                 scale=SCALE, accum_out=s1parts[:, s:s + 1])
        s1 = sm_pool.tile([128, 1], F32, tag="s1")
        nc.vector.tensor_reduce(s1[:], s1parts[:], X, mybir.AluOpType.add)
        rc1 = sm_pool.tile([128, 1], F32, tag="rc1")
        nc.vector.reciprocal(rc1[:], s1[:])
        ssc = sm_pool.tile([128, 1], F32, tag="ssc")
        nc.vector.tensor_mul(ssc[:], rc1[:], srep[:, p:p + 1])
        if p == 0:
            tap("ed0", ed[:])
            tap("s10", s1[:])

        # --- attnT via PE transposes ---
        at = at_pool.tile([128, NT, 128], BF16, tag="at")
        for t in range(NT):
            atp = pstr.tile([128, 128], BF16, tag="pstr", name="atp")
            nc.tensor.transpose(atp[:], ed[:, t * 128:(t + 1) * 128], identb[:])
            if t % 2 == 0:
                nc.vector.tensor_copy(at[:, t, :], atp[:])
            else:
                nc.scalar.copy(at[:, t, :], atp[:])

        # --- rep_delta[qpair, d-block p] ---
        rd_ps = ps128.tile([128, 128], F32, tag="ps128")
        for t in range(NT):
            nc.tensor.matmul(rd_ps[:], at[:, t, :], w_sb[:, t, p * 128:(p + 1) * 128],
                             start=(t == 0), stop=(t == NT - 1))

        if p == 0:
            tap("at0", at[:, 0, :])
            rd_tap = sm_pool.tile([128, 128], F32, tag="rd_tap")
            nc.vector.tensor_copy(rd_tap[:], rd_ps[:])
            tap("rd0", rd_tap[:])
        # --- reph_new (block-diag, natural layout) ---
        rnat = sm_pool.tile([128, 128], BF16, tag="rnat")
        nc.vector.memset(rnat[:], 0.0)
        for h in range(2):
            r0, r1 = 64 * h, 64 * (h + 1)
            nc.vector.scalar_tensor_tensor(
                rnat[r0:r1, r0:r1], rd_ps[r0:r1, r0:r1], ssc[r0:r1, 0:1],
                rep_pair[r0:r1, p, :],
                mybir.AluOpType.mult, mybir.AluOpType.add)

        # --- reph_new^T via PE transpose ---
        rtp = pstr.tile([128, 128], BF16, tag="pstr", name="rtp")
        nc.tensor.transpose(rtp[:], rnat[:], identb[:])
        rnT = sm_pool.tile([128, 128], BF16, tag="rnT")
        nc.vector.tensor_copy(rnT[:], rtp[:])

        if p == 0:
            tap("rnat0", rnat[:])
            tap("rnT0", rnT[:])
        # --- dots2 (block-diag, symmetric) + exp + sums ---
        d2_ps = ps128.tile([128, 128], F32, tag="ps128")
        nc.tensor.matmul(d2_ps[:], rnT[:], rnT[:], start=True, stop=True)
        ed2 = sm_pool.tile([128, 128], BF16, tag="ed2")
        nc.vector.memset(ed2[:], 0.0)
        s2 = sm_pool.tile([128, 1], F32, tag="s2")
        for h in range(2):
            r0, r1 = 64 * h, 64 * (h + 1)
            nc.scalar.activation(ed2[r0:r1, r0:r1], d2_ps[r0:r1, r0:r1], Exp,
                                 scale=SCALE, accum_out=s2[r0:r1, 0:1])

        # --- xds = attn2 @ reph_new, then scale rows by 1/(s1*s2) ---
        xds_ps = ps128.tile([128, 128], F32, tag="ps128")
        nc.tensor.matmul(xds_ps[:], ed2[:], rnat[:], start=True, stop=True)
        rc2 = sm_pool.tile([128, 1], F32, tag="rc2")
        nc.vector.reciprocal(rc2[:], s2[:])
        sc = sm_pool.tile([128, 1], F32, tag="sc")
        nc.vector.tensor_mul(sc[:], rc1[:], rc2[:])
        xds = sm_pool.tile([128, 128], BF16, tag="xds")
        nc.vector.tensor_scalar_mul(xds[:], xds_ps[:], sc[:])

        if p == 0:
            tap("ed20", ed2[:])
            tap("xds0", xds[:])
        # --- upsample: x_deltaT[d-pair, n] = xds^T @ expdots ---
        for s in range(NS):
            up_ps = ps512.tile([128, 512], F32, tag="ps512")
            nc.tensor.matmul(up_ps[:], xds[:], ed[:, s * 512:(s + 1) * 512],
                             start=True, stop=True)
            nc.vector.tensor_copy(xdT[p][:, s * 512:(s + 1) * 512], up_ps[:])

    tap("xdT0", xdT[0][:])
    # ================= Phase 5: out = x_delta @ to_out_w^T + b =========
    for t in range(NT):
        ops = ps512.tile([128, 512], F32, tag="ps512")
        for di in range(CH):
            nc.tensor.matmul(ops[:], xdT[di][:, t * 128:(t + 1) * 128], twT[:, di, :],
                             start=(di == 0), stop=(di == CH - 1))
        ot = ost_pool.tile([128, C], F32, tag="ostage")
        nc.vector.tensor_add(ot[:], ops[:], bias[:])
        nc.sync.dma_start(out_d[t * 128:(t + 1) * 128, :], ot[:])


def _prep_inputs(x, proj_w, step_rep, step_x, to_out_w, to_out_b):
    x = np.asarray(x, dtype=np.float32)
    proj_w = np.asarray(proj_w, dtype=np.float32)
    step_rep = np.asarray(step_rep, dtype=np.float32).reshape(HEADS)
    step_x = np.asarray(step_x, dtype=np.float32).reshape(HEADS)
    to_out_w = np.asarray(to_out_w, dtype=np.float32)
    to_out_b = np.asarray(to_out_b, dtype=np.float32)

    pwT = np.ascontiguousarray(proj_w.T.reshape(CH, 128, C).transpose(1, 0, 2))
    twTs = np.ascontiguousarray(to_out_w.T) * np.repeat(step_x, DH)[:, None]
    twTs = np.ascontiguousarray(
        twTs.reshape(CH, 128, C).transpose(1, 0, 2)).astype(ml_dtypes.bfloat16)
    bias = np.broadcast_to(to_out_b, (128, C)).copy()

    # pooling matrix M[n, q] = 1/64 for members, layout [128, NT, Q]
    n_idx = np.arange(N)
    h_idx, w_idx = n_idx // 64, n_idx % 64
    q_idx = (h_idx // 8) * 8 + (w_idx // 8)
    M = np.zeros((N, Q), dtype=np.float32)
    M[n_idx, q_idx] = 1.0 / 64.0
    mp = M.reshape(NT, 128, Q).transpose(1, 0, 2).astype(ml_dtypes.bfloat16)
    mp = np.ascontiguousarray(mp)

    srep = np.empty((128, PAIRS), dtype=np.float32)
    for p in range(PAIRS):
        srep[0:64, p] = step_rep[2 * p]
        srep[64:128, p] = step_rep[2 * p + 1]

    identf = np.eye(128, dtype=np.float32)
    identb = np.eye(128, dtype=ml_dtypes.bfloat16)

    shared = {
        "pwT": pwT, "twT": twTs, "bias": bias, "mpool": mp,
        "srep": srep, "identf": identf, "identb": identb,
    }
    in_maps = []
    for b in range(B):
        xT = np.ascontiguousarray(x[b].T.reshape(CH, 128, N).transpose(1, 0, 2))
        in_maps.append({"xT": xT, **shared})
    return in_maps


def kernel(x, proj_w, step_rep, step_x, to_out_w, to_out_b):
    if "nc" not in _CACHE:
        _CACHE["nc"] = _build()
    nc = _CACHE["nc"]
    in_maps = _prep_inputs(x, proj_w, step_rep, step_x, to_out_w, to_out_b)
    res = bass_utils.run_bass_kernel_spmd(nc, in_maps, core_ids=list(range(B)))
    return np.stack([res.results[b]["out"] for b in range(B)], axis=0)
